# revision 6
# baseline (speedup 1.0000x reference)
"""GQA multi-head attention (B=2, T=2048, C=2048, H=32, KVH=8, HD=64) with RoPE
and causal masking, distributed over 8 Trainium2 NeuronCores.

Sharding: core c -> batch b = c//4, head-group g = c%4 (8 Q-heads + 2 KV-heads
per core; Wq/Wk/Wv column-parallel, Wo row-parallel).  Each core computes a
full [T, C] partial of the output projection in bf16; the host sums the 4
partials of each batch.

v4 (from the 393us v2 baseline; PE busy was 357us at 50% HFU):
 - Scores contract over d=64 only, so per-head score matmuls use half the PE
   array.  The (kv0-head, kv1-head) pair of each block runs as two ADJACENT
   row-tiled matmuls (tile_position (0,0)/(64,0)) which the PE executes
   concurrently (HW-verified: pair starts 4-20ns apart) -> 2x on scores.
 - P/V are fp16 (fp8 fails: attention out is itself a weighted average of v,
   so P/V quantization noise of ~3.6% lands ~1:1 on the output).
 - All PE transposes eliminated: Q/K/V reach their attention layouts via
   SBUF->SBUF DMA XBAR transposes, one batched [128,4,128] 3D-out
   instruction per 512-wide group, zero PE cost.
 - exp runs as one [128, 2x512] 2-bank ACT per k-tile (both head parities),
   halving the 352-cycle-per-instruction ACT overhead (210us -> ~163us).
 - DMA issue spread over queues: x-input + transposes on sync, attention-out
   staging on scalar, weights + final output on gpsimd (v3 serialized 224
   DMA instructions on sync = 197us queue busy and starved the PE).
 - Phases A (proj), C (attention), D (out-proj) are emission-interleaved so
   the PE never starves while ACT runs exp.
"""

import numpy as np

B, T, C = 2, 2048, 2048
H, KVH, HD = 32, 8, 64
NCORES = 8
QC = 512            # q columns per core (8 heads)
NT = T // 128       # 16 t/k tiles
NCP = C // 128      # 16 c panels
NSB = 4             # t superblocks
SBT = T // NSB      # 512
EXP_SCALE = 1.0 / np.sqrt(HD)
EXP_BIAS = -2.0     # cancels in softmax


def _build_program():
    import concourse.bass as bass
    import concourse.mybir as mybir
    import concourse.tile as tile
    from collections import deque
    from concourse import bacc
    from concourse._compat import get_trn_type
    from concourse.masks import make_upper_triangular

    F32 = mybir.dt.float32
    BF16 = mybir.dt.bfloat16
    FP16 = mybir.dt.float16
    MUL = mybir.AluOpType.mult
    ADD = mybir.AluOpType.add
    SUB = mybir.AluOpType.subtract
    EXP = mybir.ActivationFunctionType.Exp

    nc = bacc.Bacc(get_trn_type() or "TRN2", target_bir_lowering=False, debug=True)

    xt = nc.dram_tensor("xt", [C, T], BF16, kind="ExternalInput")
    wq = nc.dram_tensor("wq", [C, QC], BF16, kind="ExternalInput")
    wkv = nc.dram_tensor("wkv", [C, 256], BF16, kind="ExternalInput")
    wo = nc.dram_tensor("wo", [QC, C], BF16, kind="ExternalInput")
    csj = nc.dram_tensor("csj", [T, 64], F32, kind="ExternalInput")  # [t, cos32|sin32]
    outp = nc.dram_tensor("outp", [T, C], BF16, kind="ExternalOutput")

    def tt_op(out, in0, in1, op):
        nc.vector.tensor_tensor(out=out, in0=in0, in1=in1, op=op)

    with tile.TileContext(nc) as tc:
        with (
            tc.tile_pool(name="p1", bufs=1) as p1,
            tc.tile_pool(name="p2", bufs=2) as p2,
            tc.tile_pool(name="p3", bufs=2) as p3,
            tc.tile_pool(name="psS", bufs=2, space="PSUM") as psS,
            tc.tile_pool(name="psT", bufs=1, space="PSUM") as psT,
            tc.tile_pool(name="psAD", bufs=2, space="PSUM") as psAD,
        ):
            # ---- input DMAs: spread across both hardware queues ----
            wkv_s = p1.tile([128, NCP, 256], BF16)
            wkv_r = wkv[:].rearrange("(n p) q -> p n q", p=128)
            csj_t = p1.tile([128, NT, 64], F32)
            nc.gpsimd.dma_start(csj_t[:], csj[:].rearrange("(n p) d -> p n d", p=128))
            wq_s = p1.tile([128, NCP, QC], BF16)
            wo_s = p1.tile([128, 4, C], BF16)  # DMA deferred into gen_A(1)
            # x^T lives in SBUF for the whole kernel (64KB/partition).  The
            # t<512 slice streams now in paced 2-panel chunks on both hw
            # queues (A0 needs it); t>=512 follows in wide-descriptor bulk
            # chunks at gen_A(1) (see below).
            xts_g = p1.tile([128, NCP, T], BF16)
            xt_r = xt[:].rearrange("(n p) t -> p n t", p=128)
            wq_r = wq[:].rearrange("(n p) q -> p n q", p=128)
            for q in range(4):
                nc.sync.dma_start(wkv_s[:, 4 * q:4 * q + 4, :],
                                  wkv_r[:, 4 * q:4 * q + 4, :])
                nc.sync.dma_start(xTs0 := xts_g[:, 4 * q + 2:4 * q + 4, 0:SBT],
                                  xt_r[:, 4 * q + 2:4 * q + 4, 0:SBT])
            for q in range(4):
                nc.scalar.dma_start(xts_g[:, 4 * q:4 * q + 2, 0:SBT],
                                    xt_r[:, 4 * q:4 * q + 2, 0:SBT])
                nc.scalar.dma_start(wq_s[:, 4 * q:4 * q + 4, :],
                                    wq_r[:, 4 * q:4 * q + 4, :])

            # ---- constants ----
            mk32 = p1.tile([128, 128], F32)
            make_upper_triangular(nc, mk32[:], val=1.0, diag=True)
            maskT = p1.tile([128, 128], BF16)
            nc.vector.tensor_copy(maskT[:], mk32[:])
            biasT = p1.tile([128, 1], F32)
            nc.gpsimd.memset(biasT[:], EXP_BIAS)

            # ---- persistent activations ----
            QT = p1.tile([128, 4, T], BF16)   # block m: [hA e|o | hB e|o] x t
            KT = p1.tile([128, T], BF16)      # [kv0 (e|o) | kv1 (e|o)] x t
            ohT = p1.tile([128, 4, T], BF16)  # attention out, D layout
            Vaug = p1.tile([128, 2, NT, 68], FP16)  # [t-in-tile, kv, ktile, d|1|pad]
            nc.gpsimd.memset(Vaug[:, :, :, 64:65], 1.0)

            # warm up the exp table set during the initial DMA waits
            warm = p2.tile([128, 1], F32, tag="rr", name="warm")
            nc.scalar.activation(warm[:], biasT[:], EXP, bias=biasT[:])

            # ================= Phase A: projections + rope =================
            def gen_A(sb):
                t0 = sb * SBT
                xTs = xts_g[:, :, t0:t0 + SBT]
                if sb == 1:
                    # bulk x for sb1-3: 3KB-row descriptors, sync queue (idle
                    # again by now); wo on the gpsimd ring
                    for q in range(8):
                        nc.sync.dma_start(xts_g[:, 2 * q:2 * q + 2, SBT:T],
                                          xt_r[:, 2 * q:2 * q + 2, SBT:T])
                    nc.gpsimd.dma_start(wo_s[:], wo[:].rearrange("(m p) c -> p m c", p=128))
                yield

                def emit_q(tl):
                    tt = sb * 4 + tl
                    pq = psAD.tile([128, QC], F32, tag="ad", name=f"pq{sb}_{tl}")
                    for ci in range(NCP):
                        nc.tensor.matmul(pq[:], xTs[:, ci, tl * 128:(tl + 1) * 128],
                                         wq_s[:, ci, :],
                                         start=(ci == 0), stop=(ci == NCP - 1))
                        yield
                    qn = p2.tile([128, QC], F32, tag="qn")
                    nc.vector.tensor_copy(qn[:], pq[:])
                    qr = p2.tile([128, QC], BF16, tag="qr")
                    qn3 = qn[:].rearrange("p (h e d) -> p h e d", h=8, e=2)
                    qr3 = qr[:].rearrange("p (h e d) -> p h e d", h=8, e=2)
                    qe, qo = qn3[:, :, 0, :], qn3[:, :, 1, :]
                    qre, qro = qr3[:, :, 0, :], qr3[:, :, 1, :]
                    cj = csj_t[:, tt, 0:32]
                    sj = csj_t[:, tt, 32:64]
                    cjb = bass.AP(cj.tensor, cj.offset, [cj.ap[0], [0, 8], [1, 32]])
                    sjb = bass.AP(sj.tensor, sj.offset, [sj.ap[0], [0, 8], [1, 32]])
                    t1 = p2.tile([128, 8, 32], F32, tag="t1")
                    t2 = p2.tile([128, 8, 32], F32, tag="t2")
                    tt_op(t1[:], qe, cjb, MUL)
                    tt_op(t2[:], qo, sjb, MUL)
                    tt_op(qre, t1[:], t2[:], SUB)
                    t1b = p2.tile([128, 8, 32], F32, tag="t1")
                    t2b = p2.tile([128, 8, 32], F32, tag="t2")
                    tt_op(t1b[:], qo, cjb, MUL)
                    tt_op(t2b[:], qe, sjb, MUL)
                    tt_op(qro, t1b[:], t2b[:], ADD)
                    yield
                    nc.scalar.dma_start(QT[:, :, tt * 128:(tt + 1) * 128], qr[:],
                                        transpose=True)
                    yield

                def emit_kv():
                    # K^T / V^T panels (contract over C).  A0 interleaves K/V
                    # per panel (DMA-paced); later sbs serial (1 bank each).
                    pk = psAD.tile([128, SBT], F32, tag="ad", name=f"pk{sb}")
                    if sb == 0:
                        pv = psAD.tile([128, SBT], F32, tag="ad", name=f"pv{sb}")
                        for ci in range(NCP):
                            nc.tensor.matmul(pk[:], wkv_s[:, ci, 0:128], xTs[:, ci, :],
                                             start=(ci == 0), stop=(ci == NCP - 1))
                            yield
                            nc.tensor.matmul(pv[:], wkv_s[:, ci, 128:256], xTs[:, ci, :],
                                             start=(ci == 0), stop=(ci == NCP - 1))
                            yield
                    else:
                        for ci in range(NCP):
                            nc.tensor.matmul(pk[:], wkv_s[:, ci, 0:128], xTs[:, ci, :],
                                             start=(ci == 0), stop=(ci == NCP - 1))
                            yield
                    ktr = p2.tile([128, SBT], BF16, tag="ktr", name=f"ktr{sb}")
                    nc.vector.tensor_copy(ktr[:], pk[:])
                    if sb != 0:
                        pv = psAD.tile([128, SBT], F32, tag="ad", name=f"pv{sb}")
                        for ci in range(NCP):
                            nc.tensor.matmul(pv[:], wkv_s[:, ci, 128:256], xTs[:, ci, :],
                                             start=(ci == 0), stop=(ci == NCP - 1))
                            yield
                    vt = p2.tile([128, SBT], BF16, tag="vt", name=f"vt{sb}")
                    nc.vector.tensor_copy(vt[:], pv[:])
                    yield
                    # K rope for the whole superblock (fused [128,4,...] ops)
                    kn = p2.tile([128, 4, 128], BF16, tag="kn")
                    nc.scalar.dma_start(kn[:], ktr[:], transpose=True)
                    knr = p2.tile([128, 4, 128], BF16, tag="knr")
                    kn5 = kn[:].rearrange("p l (v e d) -> p l v e d", v=2, e=2)
                    knr5 = knr[:].rearrange("p l (v e d) -> p l v e d", v=2, e=2)
                    ke, ko = kn5[:, :, :, 0, :], kn5[:, :, :, 1, :]
                    kre, kro = knr5[:, :, :, 0, :], knr5[:, :, :, 1, :]
                    cj = csj_t[:, 4 * sb:4 * sb + 4, 0:32]
                    sj = csj_t[:, 4 * sb:4 * sb + 4, 32:64]
                    cjb = bass.AP(cj.tensor, cj.offset,
                                  [cj.ap[0], [64, 4], [0, 2], [1, 32]])
                    sjb = bass.AP(sj.tensor, sj.offset,
                                  [sj.ap[0], [64, 4], [0, 2], [1, 32]])
                    k1 = p2.tile([128, 4, 2, 32], F32, tag="k1")
                    k2 = p2.tile([128, 4, 2, 32], F32, tag="k2")
                    tt_op(k1[:], ke, cjb, MUL)
                    tt_op(k2[:], ko, sjb, MUL)
                    tt_op(kre, k1[:], k2[:], SUB)
                    k1b = p2.tile([128, 4, 2, 32], F32, tag="k1")
                    k2b = p2.tile([128, 4, 2, 32], F32, tag="k2")
                    tt_op(k1b[:], ko, cjb, MUL)
                    tt_op(k2b[:], ke, sjb, MUL)
                    tt_op(kro, k1b[:], k2b[:], ADD)
                    yield
                    ktv = KT[:, t0:t0 + SBT].rearrange("p (l t) -> p l t", l=4)
                    nc.scalar.dma_start(ktv, knr[:], transpose=True)
                    # V: transpose then fp16-cast into Vaug (with ones col)
                    vn = p2.tile([128, 4, 128], BF16, tag="vn")
                    nc.scalar.dma_start(vn[:], vt[:], transpose=True)
                    for kv in range(2):
                        nc.vector.tensor_copy(
                            Vaug[:, kv, 4 * sb:4 * sb + 4, 0:64],
                            vn[:, :, kv * 64:(kv + 1) * 64])
                    yield

                if sb == 0:
                    yield from emit_kv()
                    for tl in range(NSB):
                        yield from emit_q(tl)
                else:
                    for tl in range(NSB):
                        yield from emit_q(tl)
                    yield from emit_kv()

            # ================= Phase D: output projection =================
            def gen_D(sb, tail=False):
                for tl in range(NSB):
                    tt = sb * 4 + tl
                    ost = p2.tile([128, C], BF16, tag="ost")
                    for cc in range(4):
                        po = psAD.tile([128, 512], F32, tag="ad",
                                       name=f"po{sb}_{tl}_{cc}")
                        for m in range(4):
                            nc.tensor.matmul(po[:], ohT[:, m, tt * 128:(tt + 1) * 128],
                                             wo_s[:, m, cc * 512:(cc + 1) * 512],
                                             start=(m == 0), stop=(m == 3))
                            if m < 3:
                                yield
                        if tail and cc % 2 == 1:
                            nc.scalar.copy(ost[:, cc * 512:(cc + 1) * 512], po[:])
                        else:
                            nc.vector.tensor_copy(ost[:, cc * 512:(cc + 1) * 512], po[:])
                        if cc % 2 == 1:
                            eng = nc.sync if (tl + cc // 2) % 2 == 0 else nc.gpsimd
                            eng.dma_start(
                                outp[tt * 128:(tt + 1) * 128, (cc - 1) * 512:(cc + 1) * 512],
                                ost[:, (cc - 1) * 512:(cc + 1) * 512])
                        yield

            # ================= Phase C: attention =================
            bg = deque()

            def pump(n=1):
                for _ in range(n):
                    while bg:
                        try:
                            next(bg[0])
                            break
                        except StopIteration:
                            bg.popleft()

            def drain(gen):
                for _ in gen:
                    pass

            def emit_C(sb):
                q0 = sb * SBT
                nki = 4 * sb + 4
                for m in range(4):
                    soe = psT.tile([128, SBT], F32, tag="soe", name=f"soe{sb}_{m}")
                    soo = psT.tile([128, SBT], F32, tag="soo", name=f"soo{sb}_{m}")
                    for ki in range(nki):
                        k0 = ki * 128
                        diag = ki >= 4 * sb
                        w = q0 + SBT - k0 if diag else SBT
                        c0 = k0 - q0 if diag else 0
                        g0 = max(k0, q0)
                        ps = psS.tile([128, 2, SBT], F32, tag="sg",
                                      name=f"ps{sb}_{m}_{ki}")
                        nc.tensor.matmul(ps[:, 0, 0:w], KT[0:64, k0:k0 + 128],
                                         QT[0:64, m, g0:g0 + w],
                                         start=True, stop=True)
                        nc.tensor.matmul(ps[:, 1, 0:w], KT[64:128, k0:k0 + 128],
                                         QT[64:128, m, g0:g0 + w],
                                         start=True, stop=True)
                        pp = p3.tile([128, 2, SBT], FP16, tag="pp",
                                     name=f"pp{sb}_{m}_{ki}")
                        nc.scalar.activation(pp[:, :, 0:w], ps[:, :, 0:w],
                                             EXP, scale=float(EXP_SCALE),
                                             bias=biasT[:])
                        pump(3)
                        if diag:  # mask q<k on the diagonal square, both parities
                            mb = bass.AP(maskT[:].tensor, maskT[:].offset,
                                         [maskT[:].ap[0], [0, 2], [1, 128]])
                            tt_op(pp[:, :, 0:128], pp[:, :, 0:128], mb, MUL)
                        start = ki == 0
                        stop = ki == nki - 1
                        nc.tensor.matmul(soe[0:65, c0:c0 + w],
                                         Vaug[:, 0, ki, 0:65], pp[:, 0, 0:w],
                                         start=start, stop=stop)
                        nc.tensor.matmul(soo[0:65, c0:c0 + w],
                                         Vaug[:, 1, ki, 0:65], pp[:, 1, 0:w],
                                         start=start, stop=stop)
                        pump(3)
                    # ---- epilogue: divide by the denominator row (part 64)
                    for par, sou in ((0, soe), (1, soo)):
                        rr = p2.tile([1, SBT], F32, tag="rr")
                        nc.vector.tensor_copy(rr[0:1, :], sou[64:65, :])
                        rv = p2.tile([1, SBT], F32, tag="rv")
                        nc.vector.reciprocal_approx_fast(rv[0:1, :], rr[0:1, :])
                        rp = p2.tile([64, SBT], F32, tag="rp")
                        nc.gpsimd.partition_broadcast(rp[:], rv[0:1, :], channels=64)
                        base = 0 if par == 0 else 64
                        tt_op(ohT[base:base + 64, m, q0:q0 + SBT],
                              sou[0:64, :], rp[:], MUL)
                        pump(2)

            gens_a = [gen_A(sb) for sb in range(NSB)]
            gens_d = [gen_D(0), gen_D(1), gen_D(2), gen_D(3, tail=True)]
            with nc.named_scope("phaseA0"):
                drain(gens_a[0])
            d_sched = {2: [0], 3: [1, 2]}
            for sb in range(NSB):
                with nc.named_scope(f"phaseC{sb}"):
                    if sb > 0:
                        drain(gens_a[sb])  # force-finish A(sb) before C(sb)
                    if sb < NSB - 1:
                        bg.appendleft(gens_a[sb + 1])  # A fillers before D fillers
                    for di in d_sched.get(sb, []):
                        bg.append(gens_d[di])
                    emit_C(sb)
            with nc.named_scope("phaseTail"):
                bg.append(gens_d[3])
                while bg:
                    try:
                        next(bg[0])
                    except StopIteration:
                        bg.popleft()

    nc.finalize()
    return nc


_RUNNER = None


def _get_runner():
    """Build the program once and return a cached jitted 8-core runner."""
    global _RUNNER
    if _RUNNER is not None:
        return _RUNNER

    import jax
    import concourse.mybir as mybir
    from concourse import bass2jax
    from jax.experimental.shard_map import shard_map
    from jax.sharding import Mesh, PartitionSpec

    nc = _build_program()
    bass2jax.install_neuronx_cc_hook()

    partition_name = nc.partition_id_tensor.name if nc.partition_id_tensor else None
    in_names, out_names, out_avals, zero_outs = [], [], [], []
    for alloc in nc.m.functions[0].allocations:
        if not isinstance(alloc, mybir.MemoryLocationSet):
            continue
        name = alloc.memorylocations[0].name
        if alloc.kind == "ExternalInput":
            if name != partition_name:
                in_names.append(name)
        elif alloc.kind == "ExternalOutput":
            shape = tuple(alloc.tensor_shape)
            dtype = mybir.dt.np(alloc.dtype)
            out_names.append(name)
            out_avals.append(jax.core.ShapedArray(shape, dtype))
            zero_outs.append(np.zeros(shape, dtype))
    n_params = len(in_names)
    n_outs = len(out_avals)
    all_names = list(in_names) + list(out_names)
    if partition_name is not None:
        all_names.append(partition_name)
    donate = tuple(range(n_params, n_params + n_outs))

    def _body(*args):
        operands = list(args)
        if partition_name is not None:
            operands.append(bass2jax.partition_id_tensor())
        outs = bass2jax._bass_exec_p.bind(
            *operands,
            out_avals=tuple(out_avals),
            in_names=tuple(all_names),
            out_names=tuple(out_names),
            lowering_input_output_aliases=(),
            sim_require_finite=True,
            sim_require_nnan=True,
            nc=nc,
        )
        return tuple(outs)

    devices = jax.devices()[:NCORES]
    mesh = Mesh(np.asarray(devices), ("core",))
    sharded = jax.jit(
        shard_map(_body, mesh=mesh,
                  in_specs=(PartitionSpec("core"),) * (n_params + n_outs),
                  out_specs=(PartitionSpec("core"),) * n_outs,
                  check_rep=False),
        donate_argnums=donate, keep_unused=True,
    )

    def run(in_maps):
        if nc.dbg_addr is not None:
            # No BassDebugger under axon; a zero PA makes the debug guard skip.
            dbg = np.zeros((1, 2), np.uint32)
            in_maps = [{**m, nc.dbg_addr.name: dbg} for m in in_maps]
        concat_in = [
            np.concatenate([np.asarray(in_maps[c][name]) for c in range(NCORES)], axis=0)
            for name in in_names
        ]
        concat_zeros = [np.zeros((NCORES * z.shape[0], *z.shape[1:]), z.dtype)
                        for z in zero_outs]
        out_arrs = sharded(*concat_in, *concat_zeros)
        return [
            {name: np.asarray(out_arrs[i]).reshape(NCORES, *out_avals[i].shape)[c]
             for i, name in enumerate(out_names)}
            for c in range(NCORES)
        ]

    _RUNNER = run
    return run


def make_in_maps(x, freq_cis, Wq, Wk, Wv, Wo):
    """Host-side sharding: per-core input dicts (all heavy tensors in bf16)."""
    import ml_dtypes
    bf16 = ml_dtypes.bfloat16

    x = np.asarray(x, np.float32)
    freq_cis = np.asarray(freq_cis, np.float32)
    Wq, Wk, Wv, Wo = (np.asarray(a, np.float32) for a in (Wq, Wk, Wv, Wo))

    cos, sin = freq_cis[:, :, 0], freq_cis[:, :, 1]            # [T, 32]
    csj = np.ascontiguousarray(np.concatenate([cos, sin], axis=1))  # [T, 64]

    dperm = np.concatenate([np.arange(0, HD, 2), np.arange(1, HD, 2)])  # evens|odds
    xts = [np.ascontiguousarray(x[b].T.astype(bf16)) for b in range(B)]
    in_maps = []
    for c in range(NCORES):
        b, g = divmod(c, 4)
        # head slot s -> global head: even slots from kv0's 4 heads,
        # odd slots from kv1's 4 heads (partition base 64*(s%2) everywhere)
        gheads = [g * 8 + (s // 2) + 4 * (s % 2) for s in range(8)]
        qcols = np.concatenate([gh * HD + dperm for gh in gheads])
        kcols = np.concatenate([(2 * g + kv) * HD + dperm for kv in range(2)])
        vcols = np.arange(2 * g * HD, (2 * g + 2) * HD)
        worows = np.concatenate([gh * HD + np.arange(HD) for gh in gheads])
        in_maps.append({
            "xt": xts[b],
            "wq": np.ascontiguousarray(Wq[:, qcols].astype(bf16)),
            "wkv": np.ascontiguousarray(
                np.concatenate([Wk[:, kcols], Wv[:, vcols]], axis=1).astype(bf16)),
            "wo": np.ascontiguousarray(Wo[worows, :].astype(bf16)),
            "csj": csj,
        })
    return in_maps


def combine_outputs(results):
    """Sum the 4 row-parallel bf16 partials of each batch."""
    out = np.zeros((B, T, C), np.float32)
    for c in range(NCORES):
        b = c // 4
        out[b] += np.asarray(results[c]["outp"]).astype(np.float32)
    return out


def kernel(x, freq_cis, mask, window, Wq, Wk, Wv, Wo):
    run = _get_runner()
    in_maps = make_in_maps(x, freq_cis, Wq, Wk, Wv, Wo)
    results = run(in_maps)
    return combine_outputs(results)


# revision 7
# speedup vs baseline: 1.1902x; 1.1902x over previous
"""GQA multi-head attention (B=2, T=2048, C=2048, H=32, KVH=8, HD=64) with RoPE
and causal masking, distributed over 8 Trainium2 NeuronCores.

Sharding: core c -> batch b = c//4, head-group g = c%4 (8 Q-heads + 2 KV-heads
per core; Wq/Wk/Wv column-parallel, Wo row-parallel).  Each core computes a
full [T, C] partial of the output projection in bf16; the host sums the 4
partials of each batch.

v4 (from the 393us v2 baseline; PE busy was 357us at 50% HFU):
 - Scores contract over d=64 only, so per-head score matmuls use half the PE
   array.  The (kv0-head, kv1-head) pair of each block runs as two ADJACENT
   row-tiled matmuls (tile_position (0,0)/(64,0)) which the PE executes
   concurrently (HW-verified: pair starts 4-20ns apart) -> 2x on scores.
 - P/V are fp16 (fp8 fails: attention out is itself a weighted average of v,
   so P/V quantization noise of ~3.6% lands ~1:1 on the output).
 - All PE transposes eliminated: Q/K/V reach their attention layouts via
   SBUF->SBUF DMA XBAR transposes, one batched [128,4,128] 3D-out
   instruction per 512-wide group, zero PE cost.
 - exp runs as one [128, 2x512] 2-bank ACT per k-tile (both head parities),
   halving the 352-cycle-per-instruction ACT overhead (210us -> ~163us).
 - DMA issue spread over queues: x-input + transposes on sync, attention-out
   staging on scalar, weights + final output on gpsimd (v3 serialized 224
   DMA instructions on sync = 197us queue busy and starved the PE).
 - Phases A (proj), C (attention), D (out-proj) are emission-interleaved so
   the PE never starves while ACT runs exp.
"""

import numpy as np

B, T, C = 2, 2048, 2048
H, KVH, HD = 32, 8, 64
NCORES = 8
QC = 512            # q columns per core (8 heads)
NT = T // 128       # 16 t/k tiles
NCP = C // 128      # 16 c panels
NSB = 4             # t superblocks
SBT = T // NSB      # 512
EXP_SCALE = 1.0 / np.sqrt(HD)
EXP_BIAS = -2.0     # cancels in softmax


def _build_program():
    import concourse.bass as bass
    import concourse.mybir as mybir
    import concourse.tile as tile
    from collections import deque
    from concourse import bacc
    from concourse._compat import get_trn_type
    from concourse.masks import make_upper_triangular

    F32 = mybir.dt.float32
    BF16 = mybir.dt.bfloat16
    FP16 = mybir.dt.float16
    MUL = mybir.AluOpType.mult
    ADD = mybir.AluOpType.add
    SUB = mybir.AluOpType.subtract
    EXP = mybir.ActivationFunctionType.Exp

    nc = bacc.Bacc(get_trn_type() or "TRN2", target_bir_lowering=False, debug=True)

    xt = nc.dram_tensor("xt", [C, T], BF16, kind="ExternalInput")
    wq = nc.dram_tensor("wq", [C, QC], BF16, kind="ExternalInput")
    wkv = nc.dram_tensor("wkv", [C, 256], BF16, kind="ExternalInput")
    wo = nc.dram_tensor("wo", [QC, C], BF16, kind="ExternalInput")
    csj = nc.dram_tensor("csj", [T, 64], F32, kind="ExternalInput")  # [t, cos32|sin32]
    outp = nc.dram_tensor("outp", [T, C], BF16, kind="ExternalOutput")

    def tt_op(out, in0, in1, op):
        nc.vector.tensor_tensor(out=out, in0=in0, in1=in1, op=op)

    with tile.TileContext(nc) as tc:
        with (
            tc.tile_pool(name="p1", bufs=1) as p1,
            tc.tile_pool(name="p2", bufs=2) as p2,
            tc.tile_pool(name="p3", bufs=2) as p3,
            tc.tile_pool(name="psS", bufs=2, space="PSUM") as psS,
            tc.tile_pool(name="psT", bufs=1, space="PSUM") as psT,
            tc.tile_pool(name="psAD", bufs=2, space="PSUM") as psAD,
        ):
            # ---- input DMAs: spread across both hardware queues ----
            wkv_s = p1.tile([128, NCP, 256], BF16)
            wkv_r = wkv[:].rearrange("(n p) q -> p n q", p=128)
            csj_t = p1.tile([128, NT, 64], F32)
            nc.gpsimd.dma_start(csj_t[:], csj[:].rearrange("(n p) d -> p n d", p=128))
            wq_s = p1.tile([128, NCP, QC], BF16)
            wo_s = p1.tile([128, 4, C], BF16)  # DMA deferred into gen_A(1)
            # x^T lives in SBUF for the whole kernel (64KB/partition).  The
            # t<512 slice streams now in paced 2-panel chunks on both hw
            # queues (A0 needs it); t>=512 follows in wide-descriptor bulk
            # chunks at gen_A(1) (see below).
            xts_g = p1.tile([128, NCP, T], BF16)
            xt_r = xt[:].rearrange("(n p) t -> p n t", p=128)
            wq_r = wq[:].rearrange("(n p) q -> p n q", p=128)
            for q in range(4):
                nc.sync.dma_start(wkv_s[:, 4 * q:4 * q + 4, :],
                                  wkv_r[:, 4 * q:4 * q + 4, :])
                nc.sync.dma_start(xTs0 := xts_g[:, 4 * q + 2:4 * q + 4, 0:SBT],
                                  xt_r[:, 4 * q + 2:4 * q + 4, 0:SBT])
            for q in range(4):
                nc.scalar.dma_start(xts_g[:, 4 * q:4 * q + 2, 0:SBT],
                                    xt_r[:, 4 * q:4 * q + 2, 0:SBT])
                nc.scalar.dma_start(wq_s[:, 4 * q:4 * q + 4, :],
                                    wq_r[:, 4 * q:4 * q + 4, :])

            # ---- constants ----
            mk32 = p1.tile([128, 128], F32)
            make_upper_triangular(nc, mk32[:], val=1.0, diag=True)
            maskT = p1.tile([128, 128], BF16)
            nc.vector.tensor_copy(maskT[:], mk32[:])
            biasT = p1.tile([128, 1], F32)
            nc.gpsimd.memset(biasT[:], EXP_BIAS)

            # ---- persistent activations ----
            QT = p1.tile([128, 4, T], BF16)   # block m: [hA e|o | hB e|o] x t
            KT = p1.tile([128, T], BF16)      # [kv0 (e|o) | kv1 (e|o)] x t
            ohT = p1.tile([128, 4, T], BF16)  # attention out, D layout
            Vaug = p1.tile([128, 2, NT, 68], FP16)  # [t-in-tile, kv, ktile, d|1|pad]
            nc.gpsimd.memset(Vaug[:, :, :, 64:65], 1.0)

            # warm up the exp table set during the initial DMA waits
            warm = p2.tile([128, 1], F32, tag="rr", name="warm")
            nc.scalar.activation(warm[:], biasT[:], EXP, bias=biasT[:])

            # ================= Phase A: projections + rope =================
            def gen_A(sb):
                t0 = sb * SBT
                xTs = xts_g[:, :, t0:t0 + SBT]
                if sb == 1:
                    # bulk x: sb1's slice on sync (needed soon), sb2+3 on the
                    # gpsimd ring (latency-tolerant); wo on the ring too
                    for q in range(8):
                        nc.sync.dma_start(xts_g[:, 2 * q:2 * q + 2, SBT:2 * SBT],
                                          xt_r[:, 2 * q:2 * q + 2, SBT:2 * SBT])
                    for q in range(8):
                        nc.gpsimd.dma_start(xts_g[:, 2 * q:2 * q + 2, 2 * SBT:T],
                                            xt_r[:, 2 * q:2 * q + 2, 2 * SBT:T])
                    nc.gpsimd.dma_start(wo_s[:], wo[:].rearrange("(m p) c -> p m c", p=128))
                yield

                def emit_q(tl):
                    tt = sb * 4 + tl
                    pq = psAD.tile([128, QC], F32, tag="ad", name=f"pq{sb}_{tl}")
                    for ci in range(NCP):
                        nc.tensor.matmul(pq[:], xTs[:, ci, tl * 128:(tl + 1) * 128],
                                         wq_s[:, ci, :],
                                         start=(ci == 0), stop=(ci == NCP - 1))
                        yield
                    qn = p2.tile([128, QC], F32, tag="qn")
                    nc.vector.tensor_copy(qn[:], pq[:])
                    qr = p2.tile([128, QC], BF16, tag="qr")
                    qn3 = qn[:].rearrange("p (h e d) -> p h e d", h=8, e=2)
                    qr3 = qr[:].rearrange("p (h e d) -> p h e d", h=8, e=2)
                    qe, qo = qn3[:, :, 0, :], qn3[:, :, 1, :]
                    qre, qro = qr3[:, :, 0, :], qr3[:, :, 1, :]
                    cj = csj_t[:, tt, 0:32]
                    sj = csj_t[:, tt, 32:64]
                    cjb = bass.AP(cj.tensor, cj.offset, [cj.ap[0], [0, 8], [1, 32]])
                    sjb = bass.AP(sj.tensor, sj.offset, [sj.ap[0], [0, 8], [1, 32]])
                    t1 = p2.tile([128, 8, 32], F32, tag="t1")
                    t2 = p2.tile([128, 8, 32], F32, tag="t2")
                    tt_op(t1[:], qe, cjb, MUL)
                    tt_op(t2[:], qo, sjb, MUL)
                    tt_op(qre, t1[:], t2[:], SUB)
                    t1b = p2.tile([128, 8, 32], F32, tag="t1")
                    t2b = p2.tile([128, 8, 32], F32, tag="t2")
                    tt_op(t1b[:], qo, cjb, MUL)
                    tt_op(t2b[:], qe, sjb, MUL)
                    tt_op(qro, t1b[:], t2b[:], ADD)
                    yield
                    nc.sync.dma_start(QT[:, :, tt * 128:(tt + 1) * 128], qr[:],
                                      transpose=True)
                    yield

                def emit_kv():
                    # K^T / V^T panels (contract over C).  A0 interleaves K/V
                    # per panel (DMA-paced); later sbs serial (1 bank each).
                    pk = psAD.tile([128, SBT], F32, tag="ad", name=f"pk{sb}")
                    if sb == 0:
                        pv = psAD.tile([128, SBT], F32, tag="ad", name=f"pv{sb}")
                        for ci in range(NCP):
                            nc.tensor.matmul(pk[:], wkv_s[:, ci, 0:128], xTs[:, ci, :],
                                             start=(ci == 0), stop=(ci == NCP - 1))
                            yield
                            nc.tensor.matmul(pv[:], wkv_s[:, ci, 128:256], xTs[:, ci, :],
                                             start=(ci == 0), stop=(ci == NCP - 1))
                            yield
                    else:
                        for ci in range(NCP):
                            nc.tensor.matmul(pk[:], wkv_s[:, ci, 0:128], xTs[:, ci, :],
                                             start=(ci == 0), stop=(ci == NCP - 1))
                            yield
                    ktr = p2.tile([128, SBT], BF16, tag="ktr", name=f"ktr{sb}")
                    nc.vector.tensor_copy(ktr[:], pk[:])
                    if sb != 0:
                        pv = psAD.tile([128, SBT], F32, tag="ad", name=f"pv{sb}")
                        for ci in range(NCP):
                            nc.tensor.matmul(pv[:], wkv_s[:, ci, 128:256], xTs[:, ci, :],
                                             start=(ci == 0), stop=(ci == NCP - 1))
                            yield
                    vt = p2.tile([128, SBT], BF16, tag="vt", name=f"vt{sb}")
                    nc.vector.tensor_copy(vt[:], pv[:])
                    yield
                    # K rope for the whole superblock (fused [128,4,...] ops)
                    kn = p2.tile([128, 4, 128], BF16, tag="kn")
                    nc.sync.dma_start(kn[:], ktr[:], transpose=True)
                    knr = p2.tile([128, 4, 128], BF16, tag="knr")
                    kn5 = kn[:].rearrange("p l (v e d) -> p l v e d", v=2, e=2)
                    knr5 = knr[:].rearrange("p l (v e d) -> p l v e d", v=2, e=2)
                    ke, ko = kn5[:, :, :, 0, :], kn5[:, :, :, 1, :]
                    kre, kro = knr5[:, :, :, 0, :], knr5[:, :, :, 1, :]
                    cj = csj_t[:, 4 * sb:4 * sb + 4, 0:32]
                    sj = csj_t[:, 4 * sb:4 * sb + 4, 32:64]
                    cjb = bass.AP(cj.tensor, cj.offset,
                                  [cj.ap[0], [64, 4], [0, 2], [1, 32]])
                    sjb = bass.AP(sj.tensor, sj.offset,
                                  [sj.ap[0], [64, 4], [0, 2], [1, 32]])
                    k1 = p2.tile([128, 4, 2, 32], F32, tag="k1")
                    k2 = p2.tile([128, 4, 2, 32], F32, tag="k2")
                    tt_op(k1[:], ke, cjb, MUL)
                    tt_op(k2[:], ko, sjb, MUL)
                    tt_op(kre, k1[:], k2[:], SUB)
                    k1b = p2.tile([128, 4, 2, 32], F32, tag="k1")
                    k2b = p2.tile([128, 4, 2, 32], F32, tag="k2")
                    tt_op(k1b[:], ko, cjb, MUL)
                    tt_op(k2b[:], ke, sjb, MUL)
                    tt_op(kro, k1b[:], k2b[:], ADD)
                    yield
                    ktv = KT[:, t0:t0 + SBT].rearrange("p (l t) -> p l t", l=4)
                    nc.sync.dma_start(ktv, knr[:], transpose=True)
                    # V: transpose then fp16-cast into Vaug (with ones col)
                    vn = p2.tile([128, 4, 128], BF16, tag="vn")
                    nc.sync.dma_start(vn[:], vt[:], transpose=True)
                    for kv in range(2):
                        nc.vector.tensor_copy(
                            Vaug[:, kv, 4 * sb:4 * sb + 4, 0:64],
                            vn[:, :, kv * 64:(kv + 1) * 64])
                    yield

                if sb == 0:
                    yield from emit_kv()
                    for tl in range(NSB):
                        yield from emit_q(tl)
                else:
                    for tl in range(NSB):
                        yield from emit_q(tl)
                    yield from emit_kv()

            # ================= Phase D: output projection =================
            def gen_D(sb, tail=False):
                for tl in range(NSB):
                    tt = sb * 4 + tl
                    ost = p2.tile([128, C], BF16, tag="ost")
                    for cc in range(4):
                        po = psAD.tile([128, 512], F32, tag="ad",
                                       name=f"po{sb}_{tl}_{cc}")
                        for m in range(4):
                            nc.tensor.matmul(po[:], ohT[:, m, tt * 128:(tt + 1) * 128],
                                             wo_s[:, m, cc * 512:(cc + 1) * 512],
                                             start=(m == 0), stop=(m == 3))
                            if m < 3:
                                yield
                        if tail and cc % 2 == 1:
                            nc.scalar.copy(ost[:, cc * 512:(cc + 1) * 512], po[:])
                        else:
                            nc.vector.tensor_copy(ost[:, cc * 512:(cc + 1) * 512], po[:])
                        if cc % 2 == 1:
                            eng = nc.sync if (tl + cc // 2) % 2 == 0 else nc.gpsimd
                            eng.dma_start(
                                outp[tt * 128:(tt + 1) * 128, (cc - 1) * 512:(cc + 1) * 512],
                                ost[:, (cc - 1) * 512:(cc + 1) * 512])
                        yield

            # ================= Phase C: attention =================
            bg = deque()

            def pump(n=1):
                for _ in range(n):
                    while bg:
                        try:
                            next(bg[0])
                            break
                        except StopIteration:
                            bg.popleft()

            def drain(gen):
                for _ in gen:
                    pass

            def emit_C(sb):
                q0 = sb * SBT
                nki = 4 * sb + 4
                for m in range(4):
                    soe = psT.tile([128, SBT], F32, tag="soe", name=f"soe{sb}_{m}")
                    soo = psT.tile([128, SBT], F32, tag="soo", name=f"soo{sb}_{m}")
                    for ki in range(nki):
                        k0 = ki * 128
                        diag = ki >= 4 * sb
                        w = q0 + SBT - k0 if diag else SBT
                        c0 = k0 - q0 if diag else 0
                        g0 = max(k0, q0)
                        ps = psS.tile([128, 2, SBT], F32, tag="sg",
                                      name=f"ps{sb}_{m}_{ki}")
                        nc.tensor.matmul(ps[:, 0, 0:w], KT[0:64, k0:k0 + 128],
                                         QT[0:64, m, g0:g0 + w],
                                         start=True, stop=True)
                        nc.tensor.matmul(ps[:, 1, 0:w], KT[64:128, k0:k0 + 128],
                                         QT[64:128, m, g0:g0 + w],
                                         start=True, stop=True)
                        pp = p3.tile([128, 2, SBT], FP16, tag="pp",
                                     name=f"pp{sb}_{m}_{ki}")
                        nc.scalar.activation(pp[:, :, 0:w], ps[:, :, 0:w],
                                             EXP, scale=float(EXP_SCALE),
                                             bias=biasT[:])
                        pump(3)
                        if diag:  # mask q<k on the diagonal square, both parities
                            mb = bass.AP(maskT[:].tensor, maskT[:].offset,
                                         [maskT[:].ap[0], [0, 2], [1, 128]])
                            tt_op(pp[:, :, 0:128], pp[:, :, 0:128], mb, MUL)
                        start = ki == 0
                        stop = ki == nki - 1
                        nc.tensor.matmul(soe[0:65, c0:c0 + w],
                                         Vaug[:, 0, ki, 0:65], pp[:, 0, 0:w],
                                         start=start, stop=stop)
                        nc.tensor.matmul(soo[0:65, c0:c0 + w],
                                         Vaug[:, 1, ki, 0:65], pp[:, 1, 0:w],
                                         start=start, stop=stop)
                        pump(3)
                    # ---- epilogue: divide by the denominator row (part 64)
                    for par, sou in ((0, soe), (1, soo)):
                        rr = p2.tile([1, SBT], F32, tag="rr")
                        nc.vector.tensor_copy(rr[0:1, :], sou[64:65, :])
                        rv = p2.tile([1, SBT], F32, tag="rv")
                        nc.vector.reciprocal_approx_fast(rv[0:1, :], rr[0:1, :])
                        rp = p2.tile([64, SBT], F32, tag="rp")
                        nc.gpsimd.partition_broadcast(rp[:], rv[0:1, :], channels=64)
                        base = 0 if par == 0 else 64
                        tt_op(ohT[base:base + 64, m, q0:q0 + SBT],
                              sou[0:64, :], rp[:], MUL)
                        pump(2)

            gens_a = [gen_A(sb) for sb in range(NSB)]
            gens_d = [gen_D(0), gen_D(1), gen_D(2), gen_D(3, tail=True)]
            with nc.named_scope("phaseA0"):
                drain(gens_a[0])
            d_sched = {2: [0], 3: [1, 2]}
            for sb in range(NSB):
                with nc.named_scope(f"phaseC{sb}"):
                    if sb > 0:
                        drain(gens_a[sb])  # force-finish A(sb) before C(sb)
                    if sb < NSB - 1:
                        bg.appendleft(gens_a[sb + 1])  # A fillers before D fillers
                    for di in d_sched.get(sb, []):
                        bg.append(gens_d[di])
                    emit_C(sb)
            with nc.named_scope("phaseTail"):
                bg.append(gens_d[3])
                while bg:
                    try:
                        next(bg[0])
                    except StopIteration:
                        bg.popleft()

    nc.finalize()
    return nc


_RUNNER = None


def _get_runner():
    """Build the program once and return a cached jitted 8-core runner."""
    global _RUNNER
    if _RUNNER is not None:
        return _RUNNER

    import jax
    import concourse.mybir as mybir
    from concourse import bass2jax
    from jax.experimental.shard_map import shard_map
    from jax.sharding import Mesh, PartitionSpec

    nc = _build_program()
    bass2jax.install_neuronx_cc_hook()

    partition_name = nc.partition_id_tensor.name if nc.partition_id_tensor else None
    in_names, out_names, out_avals, zero_outs = [], [], [], []
    for alloc in nc.m.functions[0].allocations:
        if not isinstance(alloc, mybir.MemoryLocationSet):
            continue
        name = alloc.memorylocations[0].name
        if alloc.kind == "ExternalInput":
            if name != partition_name:
                in_names.append(name)
        elif alloc.kind == "ExternalOutput":
            shape = tuple(alloc.tensor_shape)
            dtype = mybir.dt.np(alloc.dtype)
            out_names.append(name)
            out_avals.append(jax.core.ShapedArray(shape, dtype))
            zero_outs.append(np.zeros(shape, dtype))
    n_params = len(in_names)
    n_outs = len(out_avals)
    all_names = list(in_names) + list(out_names)
    if partition_name is not None:
        all_names.append(partition_name)
    donate = tuple(range(n_params, n_params + n_outs))

    def _body(*args):
        operands = list(args)
        if partition_name is not None:
            operands.append(bass2jax.partition_id_tensor())
        outs = bass2jax._bass_exec_p.bind(
            *operands,
            out_avals=tuple(out_avals),
            in_names=tuple(all_names),
            out_names=tuple(out_names),
            lowering_input_output_aliases=(),
            sim_require_finite=True,
            sim_require_nnan=True,
            nc=nc,
        )
        return tuple(outs)

    devices = jax.devices()[:NCORES]
    mesh = Mesh(np.asarray(devices), ("core",))
    sharded = jax.jit(
        shard_map(_body, mesh=mesh,
                  in_specs=(PartitionSpec("core"),) * (n_params + n_outs),
                  out_specs=(PartitionSpec("core"),) * n_outs,
                  check_rep=False),
        donate_argnums=donate, keep_unused=True,
    )

    def run(in_maps):
        if nc.dbg_addr is not None:
            # No BassDebugger under axon; a zero PA makes the debug guard skip.
            dbg = np.zeros((1, 2), np.uint32)
            in_maps = [{**m, nc.dbg_addr.name: dbg} for m in in_maps]
        concat_in = [
            np.concatenate([np.asarray(in_maps[c][name]) for c in range(NCORES)], axis=0)
            for name in in_names
        ]
        concat_zeros = [np.zeros((NCORES * z.shape[0], *z.shape[1:]), z.dtype)
                        for z in zero_outs]
        out_arrs = sharded(*concat_in, *concat_zeros)
        return [
            {name: np.asarray(out_arrs[i]).reshape(NCORES, *out_avals[i].shape)[c]
             for i, name in enumerate(out_names)}
            for c in range(NCORES)
        ]

    _RUNNER = run
    return run


def make_in_maps(x, freq_cis, Wq, Wk, Wv, Wo):
    """Host-side sharding: per-core input dicts (all heavy tensors in bf16)."""
    import ml_dtypes
    bf16 = ml_dtypes.bfloat16

    x = np.asarray(x, np.float32)
    freq_cis = np.asarray(freq_cis, np.float32)
    Wq, Wk, Wv, Wo = (np.asarray(a, np.float32) for a in (Wq, Wk, Wv, Wo))

    cos, sin = freq_cis[:, :, 0], freq_cis[:, :, 1]            # [T, 32]
    csj = np.ascontiguousarray(np.concatenate([cos, sin], axis=1))  # [T, 64]

    dperm = np.concatenate([np.arange(0, HD, 2), np.arange(1, HD, 2)])  # evens|odds
    xts = [np.ascontiguousarray(x[b].T.astype(bf16)) for b in range(B)]
    in_maps = []
    for c in range(NCORES):
        b, g = divmod(c, 4)
        # head slot s -> global head: even slots from kv0's 4 heads,
        # odd slots from kv1's 4 heads (partition base 64*(s%2) everywhere)
        gheads = [g * 8 + (s // 2) + 4 * (s % 2) for s in range(8)]
        qcols = np.concatenate([gh * HD + dperm for gh in gheads])
        kcols = np.concatenate([(2 * g + kv) * HD + dperm for kv in range(2)])
        vcols = np.arange(2 * g * HD, (2 * g + 2) * HD)
        worows = np.concatenate([gh * HD + np.arange(HD) for gh in gheads])
        in_maps.append({
            "xt": xts[b],
            "wq": np.ascontiguousarray(Wq[:, qcols].astype(bf16)),
            "wkv": np.ascontiguousarray(
                np.concatenate([Wk[:, kcols], Wv[:, vcols]], axis=1).astype(bf16)),
            "wo": np.ascontiguousarray(Wo[worows, :].astype(bf16)),
            "csj": csj,
        })
    return in_maps


def combine_outputs(results):
    """Sum the 4 row-parallel bf16 partials of each batch."""
    out = np.zeros((B, T, C), np.float32)
    for c in range(NCORES):
        b = c // 4
        out[b] += np.asarray(results[c]["outp"]).astype(np.float32)
    return out


def kernel(x, freq_cis, mask, window, Wq, Wk, Wv, Wo):
    run = _get_runner()
    in_maps = make_in_maps(x, freq_cis, Wq, Wk, Wv, Wo)
    results = run(in_maps)
    return combine_outputs(results)
